# revision 46
# baseline (speedup 1.0000x reference)
"""Trainium2 Bass kernel for nn_Attention (dense transformer block attention).

Reference computation (per batch element b, fp32):
    qkv = x @ Wqkv.T; q, k, v -> heads (H=16, dh=64)
    dots = (q @ k.T) * D**-0.5; pair-masked softmax; out = attn @ v
    y = out @ Wout.T + bout
SCALE = D**-0.5 = 1/32 makes the logits tiny (std ~0.1), so q/k tolerate
aggressive quantization: fp8(e4m3) q/k perturbs attention weights <1%.

Sharding: pure batch data-parallelism. B == 8 == n_cores; each NeuronCore
computes one batch element end to end. No collectives.

Device algorithm per core:
  Phase A: q,k projection in fp8 DoubleRow matmuls (K=256/instr, 0.5
           cyc/row): PSUM fp32 -> DVE fp8 staging [128 dims, N] -> 4 shuffle
           DMAs repartition each staging tile into the DoubleRow operand
           layout [32 dims, 2 slabs, N] (4 heads per 128-partition tile).
           Weights live in SBUF whole (fp8 q/k, bf16 v/out), prearranged on
           the host into k-paired DMA-dense layouts; x arrives both fp8
           (k-paired) and bf16.
  Phase B: per head pair: scoresT[j,i] = k_h^T q_h as fp8 DoubleRow with
           32x2 contraction at partition base 32*(h%4); one Exp ACT op per
           [128, N] score tile (logits tiny: no row-max); AV in bf16 with a
           per-pair v_all block [v_even(64) | 1 | v_odd(64)] so the even
           head window [v_e|1] yields dims at partitions 0:64 + denominator
           at 64, and the odd window (offset+1, 128 wide) yields junk 0:63,
           denominator at 63, dims at 64:128 -- both PSUM-partition-aligned
           with attn_outT, eliminating staging DMAs. Normalization:
           rn = 1/(den + 1e30*rowinv) (masked query rows -> rn=0), gpsimd
           partition_broadcast replicates rn across the head's 64
           partitions, DVE multiplies during the PSUM->SBUF copy, then one
           blend op adds rowinv*vmean (masked rows output the v column
           mean, matching softmax over an all-masked row).
  Phase C: y = attn_out @ Wout.T + bout with resident bf16 weights,
           ct-inner PSUM accumulation, bias folded into the PSUM->SBUF copy.

All fp8 weights are prescaled by 32 on the host (power of two, exact);
the combined 32*32 factor is divided out of the Exp scale argument.
"""

import numpy as np

N = 1024
D = 1024
H = 16
DH = 64
SCALE = float(D) ** -0.5
NEG = -1.0e30
BIG = 1.0e30
NCORES = 8
KT = 8            # bf16 k-tiles over contraction dim
KP = 4            # fp8 k-pair tiles (256 rows each)
ST = 8            # 128-row seq tiles
PAIRS = H // 2
# v_all pair block layout: [v_even(64) | 1_e | 1_o | junk(31) | v_odd(64)]
# even AV window = cols 0:65  -> out rows 0:64 dims, 64 = denominator
# odd  AV window = cols 33:161 -> out rows 0:32 junk, 32 = denominator,
#                                 rows 64:128 dims (all 32-aligned)
VBLK = 161
VODD = 97  # v_odd offset within the block

_BUILT = {}


def _build_module():
    import concourse.bacc as bacc
    import concourse.mybir as mybir
    import concourse.tile as tile

    f32 = mybir.dt.float32
    bf16 = mybir.dt.bfloat16
    fp8 = mybir.dt.float8e4
    DR = mybir.MatmulPerfMode.DoubleRow
    Add = mybir.AluOpType.add
    Mult = mybir.AluOpType.mult
    Exp = mybir.ActivationFunctionType.Exp
    AxX = mybir.AxisListType.X

    nc = bacc.Bacc("TRN2", target_bir_lowering=False, debug=False)

    x8_d = nc.dram_tensor("x8", [128, KP, 2, N], fp8, kind="ExternalInput")
    wqk8_d = nc.dram_tensor("wqk8", [128, KP, 2, 2 * D], fp8, kind="ExternalInput")
    xlo_d = nc.dram_tensor("x8lo", [128, KP, 2, N], fp8, kind="ExternalInput")
    wvh_d = nc.dram_tensor("wv8h", [128, KP, 2, D], fp8, kind="ExternalInput")
    wvl_d = nc.dram_tensor("wv8l", [128, KP, 2, D], fp8, kind="ExternalInput")
    wo_d = nc.dram_tensor("wob", [128, KT, D], bf16, kind="ExternalInput")
    bout_d = nc.dram_tensor("boutr", [1, D], f32, kind="ExternalInput")
    cb_d = nc.dram_tensor("colbias", [N, 1], f32, kind="ExternalInput")
    u8 = mybir.dt.uint8
    rinv_d = nc.dram_tensor("rinvb", [1, N], u8, kind="ExternalInput")
    y_d = nc.dram_tensor("y", [N, D], bf16, kind="ExternalOutput")

    with tile.TileContext(nc) as tc:
        with (
            tc.tile_pool(name="inp", bufs=1) as inp,
            tc.tile_pool(name="bcast", bufs=1) as bcp,
            tc.tile_pool(name="qk8", bufs=1) as qkp,
            tc.tile_pool(name="stage", bufs=4) as stp,
            tc.tile_pool(name="vatt", bufs=1) as vap,
            tc.tile_pool(name="au", bufs=2) as aup,
            tc.tile_pool(name="rn", bufs=2) as rnp,
            tc.tile_pool(name="axs", bufs=1) as axs,
            tc.tile_pool(name="ystage", bufs=2) as ysp,
            tc.tile_pool(name="dram_rn", bufs=2, space="DRAM") as drp,
            tc.tile_pool(name="main", bufs=2, space="PSUM") as mp,
        ):
            # ---- resident inputs ----
            x8s = inp.tile([128, KP, 2, N], fp8)
            wqk8s = inp.tile([128, KP, 2, 2 * D], fp8)
            x8lo = inp.tile([128, KP, 2, N], fp8)
            wv8h = inp.tile([128, KP, 2, D], fp8)
            wv8l = inp.tile([128, KP, 2, D], fp8)
            wos = inp.tile([128, KT, D], bf16)
            for kp in range(KP):
                nc.sync.dma_start(x8s[:, kp], x8_d.ap()[:, kp])
                nc.sync.dma_start(
                    wqk8s[:, kp, :, 0:D], wqk8_d.ap()[:, kp, :, 0:D]
                )
            for kp in range(KP):
                nc.sync.dma_start(
                    wqk8s[:, kp, :, D : 2 * D], wqk8_d.ap()[:, kp, :, D : 2 * D]
                )

            # ---- small constants ----
            colbias_sb = bcp.tile([128, ST], f32)
            nc.gpsimd.dma_start(
                colbias_sb[:], cb_d.ap().rearrange("(j p) o -> p (j o)", p=128)
            )
            bout_b = bcp.tile([128, D], f32)
            rinv_bb = bcp.tile([128, N], u8)

            # ---- persistent activations ----
            q8p = [qkp.tile([64, 2, N], fp8, name=f"q8p{g}", tag=f"q8p{g}")
                   for g in range(PAIRS)]
            k8p = [qkp.tile([64, 2, N], fp8, name=f"k8p{g}", tag=f"k8p{g}")
                   for g in range(PAIRS)]
            v_all = [vap.tile([128, ST * VBLK], bf16, name=f"va{s}", tag=f"va{s}")
                     for s in range(ST)]
            attn_outT = [vap.tile([128, N], bf16, name=f"aot{t}", tag=f"aot{t}")
                         for t in range(PAIRS)]
            vmean_sb = bcp.tile([128, ST], f32)

            # ================= Phase A: q/k projection (fp8 DoubleRow) ====
            # ct 0..7 -> q dims tile ct (heads 2ct, 2ct+1)
            # ct 8..15 -> k dims tile ct-8
            # interleave q/k so early pairs unblock quickly
            def emit_qkproj(ct):
                isq = ct < ST
                col0 = (ct % ST) * 128 + (0 if isq else D)
                pq = mp.tile([128, N], f32, name="pq", tag="mp")
                for c in range(4):
                    for kp in range(KP):
                        nc.tensor.matmul(
                            pq[:, c * 256 : (c + 1) * 256],
                            wqk8s[:, kp, :, col0 : col0 + 128],
                            x8s[:, kp, :, c * 256 : (c + 1) * 256],
                            start=(kp == 0),
                            stop=(kp == KP - 1),
                            perf_mode=DR,
                        )
                stg = stp.tile([128, N], fp8, name="stg", tag="stg")
                nc.vector.tensor_copy(stg[:], pq[:])
                dst = (q8p if isq else k8p)[ct % ST]
                for hh in range(2):
                    for s in range(2):
                        nc.sync.dma_start(
                            dst[32 * hh : 32 * hh + 32, s, :],
                            stg[64 * hh + 32 * s : 64 * hh + 32 * s + 32, :],
                        )

            # prologue: projections for pairs 0 and 1; the SP-queue ordering
            # puts their shuffle DMAs ahead of the xts/wvs bulk transfers
            emit_qkproj(0)
            emit_qkproj(ST)
            nc.sync.dma_start(x8lo[:], xlo_d.ap())
            nc.sync.dma_start(wv8h[:], wvh_d.ap())
            nc.sync.dma_start(wv8l[:], wvl_d.ap())
            emit_qkproj(1)
            emit_qkproj(ST + 1)
            nc.sync.dma_start(rinv_bb[:], rinv_d.ap().to_broadcast((128, N)))

            xsum_f = axs.tile([128, KT], f32)
            xsum2 = axs.tile([128, 2 * KT], bf16)

            def emit_xsum():
                for kp in range(KP):
                    for s in range(2):
                        kt = 2 * kp + s
                        nc.vector.tensor_reduce(
                            xsum_f[:, kt : kt + 1], x8s[:, kp, s, :], AxX, Add
                        )
                nc.gpsimd.tensor_scalar(
                    xsum2[:].rearrange("p (k two) -> p k two", two=2),
                    xsum_f[:, :, None].broadcast_to((128, KT, 2)),
                    1.0 / N,
                    None,
                    Mult,
                )

            # v_all ones/zero columns (after the critical prologue staging)
            for s in range(ST):
                va3 = v_all[s][:].rearrange("p (pair blk) -> p pair blk", blk=VBLK)
                nc.vector.memset(va3[:, :, DH : DH + 2], 1.0)
                nc.vector.memset(va3[:, :, DH + 2 : VODD], 0.0)

            vpool = [mp]

            def emit_v(st):
                pv = vpool[0].tile([128, N], f32, name="pv", tag="pv")
                passes = [(x8s, wv8h), (x8s, wv8l), (x8lo, wv8h)]
                for vc in range(4):
                    for pi, (xa, wb) in enumerate(passes):
                        for kp in range(KP):
                            nc.tensor.matmul(
                                pv[:, vc * 256 : (vc + 1) * 256],
                                xa[:, kp, :, st * 128 : (st + 1) * 128],
                                wb[:, kp, :, vc * 256 : (vc + 1) * 256],
                                start=(pi == 0 and kp == 0),
                                stop=(pi == 2 and kp == KP - 1),
                                perf_mode=DR,
                            )
                va3 = v_all[st][:].rearrange("p (pair blk) -> p pair blk", blk=VBLK)
                pv3 = pv[:].rearrange("p (pair x) -> p pair x", x=128)
                nc.vector.tensor_scalar(
                    va3[:, :, 0:DH], pv3[:, :, 0:DH], 1.0 / 32.0, None, Mult
                )
                nc.vector.tensor_scalar(
                    va3[:, :, VODD:VBLK], pv3[:, :, DH:128], 1.0 / 32.0, None, Mult
                )

            def emit_vmean(t):
                pm = mp.tile([128, N], f32, name="pm", tag="mp")
                steps = [(wb, kp, s) for wb in (wv8h, wv8l)
                         for kp in range(KP) for s in range(2)]
                for i, (wb, kp, s) in enumerate(steps):
                    kt = 2 * kp + s
                    nc.tensor.matmul(
                        pm[:, 0:2],
                        wb[:, kp, s, t * 128 : (t + 1) * 128],
                        xsum2[:, 2 * kt : 2 * kt + 2],
                        start=(i == 0),
                        stop=(i == len(steps) - 1),
                    )
                nc.vector.tensor_scalar(
                    vmean_sb[:, t : t + 1], pm[:, 0:1], 1.0 / 32.0, None, Mult
                )

            # ================= Phase B: attention pair loop =================
            # scores/exp emission runs a global cursor ahead of the AV
            # consumer, crossing pair boundaries
            au_store = {}

            def emit_scores(t, jt):
                for p in range(2):
                    pb = 32 * p
                    ps = mp.tile([128, N], f32, name="ps", tag="mp")
                    for c in range(4):
                        nc.tensor.matmul(
                            ps[:, c * 256 : (c + 1) * 256],
                            k8p[t][pb : pb + 32, :, jt * 128 : (jt + 1) * 128],
                            q8p[t][pb : pb + 32, :, c * 256 : (c + 1) * 256],
                            start=True,
                            stop=True,
                            perf_mode=DR,
                        )
                    au = aup.tile([128, N], bf16, name="au", tag="au", bufs=22)
                    nc.scalar.activation(
                        au[:],
                        ps[:],
                        Exp,
                        bias=colbias_sb[:, jt : jt + 1],
                        scale=SCALE / 1024.0,
                    )
                    au_store[(t, jt, p)] = au

            slots = [(t, jt) for t in range(PAIRS) for jt in range(ST)]
            cursor = 0
            while cursor < 3:
                emit_scores(*slots[cursor])
                cursor += 1

            # ---- V-projection prologue: own PSUM pool (the avs banks are
            # still free) so scores stream on mp at full ACT pace ----
            with tc.tile_pool(name="pvps", bufs=2, space="PSUM") as pvp:
                vpool[0] = pvp
                for jt in range(ST):
                    emit_v(jt)
                    if jt == 2:
                        emit_xsum()
                    for _ in range(2):
                        if cursor < 10:
                            emit_scores(*slots[cursor])
                            cursor += 1

            pending = []
            with tc.tile_pool(name="pav", bufs=1, space="PSUM") as pavp:
                for t in range(PAIRS):
                    if t == 2:
                        # behind pair-2's shuffles on the in-order SP queue:
                        # transfers only after the critical early loads
                        nc.sync.dma_start(wos[:], wo_d.ap())
                        nc.sync.dma_start(
                            bout_b[:], bout_d.ap().to_broadcast((128, D))
                        )
                    if t + 2 < PAIRS:
                        emit_qkproj(t + 2)
                        emit_qkproj(ST + t + 2)
                    avs = [
                        pavp.tile([128, N], f32, name=f"av{p}", tag=f"av{p}")
                        for p in range(2)
                    ]
                    for jt in range(ST):
                        if jt == 4:
                            emit_vmean(t)
                        base = t * VBLK
                        for p in range(2):
                            win = (
                                v_all[jt][:, base : base + DH + 1]
                                if p == 0
                                else v_all[jt][:, base + 33 : base + VBLK]
                            )
                            rows = DH + 1 if p == 0 else 128
                            au = au_store.pop((t, jt, p))
                            for sc in range(2):
                                nc.tensor.matmul(
                                    avs[p][0:rows, sc * 512 : (sc + 1) * 512],
                                    win,
                                    au[:, sc * 512 : (sc + 1) * 512],
                                    start=(jt == 0),
                                    stop=(jt == ST - 1),
                                )
                        consumed = t * ST + jt + 1
                        for _ in range(2):
                            if cursor < len(slots) and cursor - consumed < 10:
                                emit_scores(*slots[cursor])
                                cursor += 1
                    # ---- epilogue: free avs fast; the normalization of the
                    # PREVIOUS pair runs here so its reciprocal never blocks
                    # the in-order DVE queue waiting on the den broadcast DMA
                    den_st = rnp.tile([128, N], f32, name="den_st", tag="den_st")
                    den_bb = rnp.tile([128, N], f32, name="den_bb", tag="den_bb")
                    nc.vector.tensor_copy(attn_outT[t][0:DH, :], avs[0][0:DH, :])
                    nc.vector.tensor_copy(den_st[64:65, :], avs[0][DH : DH + 1, :])
                    nc.vector.tensor_copy(den_st[32:33, :], avs[1][32:33, :])
                    nc.vector.tensor_copy(attn_outT[t][DH:128, :], avs[1][DH:128, :])
                    rn_dram = drp.tile([2, N], f32)
                    nc.sync.dma_start(rn_dram[0:1, :], den_st[64:65, :])
                    nc.sync.dma_start(rn_dram[1:2, :], den_st[32:33, :])
                    nc.sync.dma_start(
                        den_bb[0:64, :], rn_dram[0:1, :].to_broadcast((64, N))
                    )
                    nc.sync.dma_start(
                        den_bb[64:128, :], rn_dram[1:2, :].to_broadcast((64, N))
                    )
                    pending.append((t, den_bb))
                    if len(pending) > 1 or t == PAIRS - 1:
                        for tp, dbb in pending[: None if t == PAIRS - 1 else -1]:
                            nc.vector.reciprocal(dbb[:], dbb[:])
                            nc.vector.tensor_tensor(
                                attn_outT[tp][:], attn_outT[tp][:], dbb[:], Mult
                            )
                            nc.vector.copy_predicated(
                                attn_outT[tp][:],
                                rinv_bb[:],
                                vmean_sb[:, tp : tp + 1].broadcast_to((128, N)),
                            )
                        pending = pending[-1:] if t != PAIRS - 1 else []

            # ================= Phase C: output projection =================
            with tc.tile_pool(name="pyx", bufs=2, space="PSUM") as pyxp:
                for st in range(ST):
                    pool = mp if st % 2 == 0 else pyxp
                    py = pool.tile(
                        [128, D], f32, name="py", tag="mp" if pool is mp else "pyx"
                    )
                    for ct in range(KT):
                        for ec in range(2):
                            nc.tensor.matmul(
                                py[:, ec * 512 : (ec + 1) * 512],
                                attn_outT[ct][:, st * 128 : (st + 1) * 128],
                                wos[:, ct, ec * 512 : (ec + 1) * 512],
                                start=(ct == 0),
                                stop=(ct == KT - 1),
                            )
                    ystage = ysp.tile([128, D], bf16, name="ys", tag="ys")
                    nc.vector.scalar_tensor_tensor(
                        ystage[:], py[:], 1.0, bout_b[:], Mult, Add
                    )
                    nc.sync.dma_start(
                        y_d.ap()[st * 128 : (st + 1) * 128, :], ystage[:]
                    )

    nc.compile()
    return nc


def get_module():
    if "nc" not in _BUILT:
        _BUILT["nc"] = _build_module()
    return _BUILT["nc"]


def make_in_maps(x, mask, Wqkv, Wout, bout):
    import ml_dtypes

    bf = ml_dtypes.bfloat16
    fp8 = ml_dtypes.float8_e4m3
    x = np.asarray(x, np.float32)
    mask = np.asarray(mask, bool)
    B = x.shape[0]

    xT = np.transpose(x, (0, 2, 1))  # [B, D, N]
    # fp8 x, k-paired: x8[b, p, kp, s, n] = xT[b, (2kp+s)*128+p, n]
    xk = np.ascontiguousarray(
        xT.reshape(B, KP, 2, 128, N).transpose(0, 3, 1, 2, 4)
    ).astype(np.float32)
    x8 = xk.astype(fp8)
    x8lo = (xk - x8.astype(np.float32)).astype(fp8)

    WT = np.asarray(Wqkv, np.float32).T  # [D, 3D]
    wqk = 32.0 * WT[:, : 2 * D]
    wqk8 = np.ascontiguousarray(
        wqk.reshape(KP, 2, 128, 2 * D).transpose(2, 0, 1, 3)
    ).astype(fp8)
    wvk = np.ascontiguousarray(
        (32.0 * WT[:, 2 * D :]).reshape(KP, 2, 128, D).transpose(2, 0, 1, 3)
    ).astype(np.float32)
    wv8h = wvk.astype(fp8)
    wv8l = (wvk - wv8h.astype(np.float32)).astype(fp8)
    woT = np.asarray(Wout, np.float32).T  # [D, D]
    wob = np.ascontiguousarray(
        woT.reshape(KT, 128, D).transpose(1, 0, 2)
    ).astype(bf)

    boutr = np.ascontiguousarray(np.asarray(bout, np.float32).reshape(1, D))
    m_full = np.concatenate([np.ones((B, 1), bool), mask], axis=1)  # [B, N]
    colbias = np.where(m_full, 0.0, NEG).astype(np.float32)
    rinvb = (~m_full).astype(np.uint8)
    return [
        {
            "x8": x8[b],
            "x8lo": x8lo[b],
            "wqk8": wqk8,
            "wv8h": wv8h,
            "wv8l": wv8l,
            "wob": wob,
            "boutr": boutr,
            "colbias": np.ascontiguousarray(colbias[b].reshape(N, 1)),
            "rinvb": np.ascontiguousarray(rinvb[b].reshape(1, N)),
        }
        for b in range(B)
    ]


def kernel(x, mask, Wqkv, Wout, bout):
    from concourse.bass_utils import run_bass_kernel_spmd

    nc = get_module()
    in_maps = make_in_maps(x, mask, Wqkv, Wout, bout)
    res = run_bass_kernel_spmd(nc, in_maps, core_ids=list(range(NCORES)))
    return np.stack(
        [res.results[b]["y"].astype(np.float32) for b in range(NCORES)], axis=0
    )
